# revision 22
# baseline (speedup 1.0000x reference)
"""MoE head (top-2 routing, swiglu MLP + vocab projection) on 8 Trainium2 cores.

Expert-parallel: one expert per NeuronCore. Routing (tiny: router scores +
top-k + stable dispatch sort) is replicated bitwise on host CPU with jax-cpu
(matching the fp32 reference); each core runs its expert's full MLP + vocab
projection over that expert's tokens.

Device layout: every matmul keeps a weight tile stationary and streams
activations as the moving operand; activations live as [feature, token] so no
on-chip transposes are needed anywhere; the top-2 gate is folded into the
activations before the vocab projection, so the host combine is a pure
scatter-add.

Phases 1 (up-proj + swiglu) and 2 (down-proj + residual) run in fp8 e4m3 with
MatmulPerfMode.DoubleRow (128x256 virtual PE array, ~1.5-1.8x the fp16 matmul
rate); their quantization error lands on y *before* the vocab projection and
is attenuated by it, staying well inside the correctness budget. Phase 3
(vocab projection, 2/3 of the FLOPs) stays fp16: a single-pass fp8 vocab
matmul puts ~2.7e-2 of direct error on the logits (budget 2e-2), and
compensated multi-pass fp8 is no faster than fp16's FWL path.

Scale algebra (all powers of two, exact):
  X8 = 2^4 x, Wup8 = 2^6 wup          -> psum p = 2^10 h
  sigmoid(2^-10 p) = sigmoid(h_gate); plc = 2^-16 p_lin = 2^-6 h_lin
  A8 = plc * (sig * p_gate) = 2^4 a   (|A8| <~ 90 < 240, no fp8 overflow)
  Wdn8 = 2^6 wdn                      -> psum = 2^10 (y - x)
  X16s = 2^10 x (fp16), Gs = 2^-10 g  -> Y16 = (psum + X16s) * Gs = y*g
"""

import os
import sys
import subprocess
import tempfile

import numpy as np
import ml_dtypes

for _p in ("/opt/trn_rl_repo",):
    if os.path.isdir(_p) and _p not in sys.path:
        sys.path.insert(0, _p)

B, S, DIM = 2, 1024, 1024
N_EXPERTS, K = 8, 2
VOCAB = 16384
HIDDEN = DIM * 8 // 3            # 2730
HID_P = 2816                     # HIDDEN padded to 22*128
NQ = HID_P // 128                # 22 row tiles of lin/gate; phase-2 k-tiles
NP2 = NQ // 2                    # 11 phase-2 DoubleRow k-pairs
KD = DIM // 128                  # 8
KP = KD // 2                     # 4 phase-1/3 k-pairs
NV = VOCAB // 128                # 128
N_CORES = 8

F8 = ml_dtypes.float8_e4m3       # TRN FP8_EXP4 (max normal 240)
S_X = 16.0                       # 2^4
S_WU = 64.0                      # 2^6
S_A = 16.0                       # 2^4
S_WD = 64.0                      # 2^6
SIG_SCALE = 1.0 / 1024.0         # 2^-10 = 1/(S_X*S_WU)
PLC_SCALE = 1.0 / 65536.0        # 2^-16 = S_A/(S_X*S_WU)^2
P2_SCALE = 1024.0                # 2^10 = S_A*S_WD

# Routing must make the same discrete top-k choices as the reference, which
# runs under jax on CPU; replicate it in a JAX_PLATFORMS=cpu subprocess
# (this process's jax backend is the axon/trn2 platform).
def _cpu_jax_env():
    env = dict(os.environ)
    env.pop("TRN_TERMINAL_POOL_IPS", None)
    env["JAX_PLATFORMS"] = "cpu"
    try:
        import jax

        sp = os.path.dirname(os.path.dirname(jax.__file__))
        env["PYTHONPATH"] = sp + os.pathsep + env.get("PYTHONPATH", "")
    except Exception:
        pass
    return env


_ROUTE_SRC = r"""
import os, sys
os.environ["JAX_PLATFORMS"] = "cpu"
import numpy as np
d = sys.argv[1]
x = np.load(os.path.join(d, "x.npy"))
wr = np.load(os.path.join(d, "wr.npy"))
import jax, jax.numpy as jnp
scores = jnp.einsum("bsd,nd->bsn", jnp.asarray(x), jnp.asarray(wr))
c, ids = jax.lax.top_k(scores, 2)
w = jax.nn.softmax(c, axis=-1)
np.save(os.path.join(d, "ids.npy"), np.asarray(ids))
np.save(os.path.join(d, "w.npy"), np.asarray(w, dtype=np.float32))
"""


def _route(x, w_router):
    try:
        with tempfile.TemporaryDirectory() as d:
            np.save(os.path.join(d, "x.npy"), np.asarray(x, np.float32))
            np.save(os.path.join(d, "wr.npy"), np.asarray(w_router, np.float32))
            src = os.path.join(d, "route.py")
            with open(src, "w") as f:
                f.write(_ROUTE_SRC)
            env = _cpu_jax_env()
            subprocess.run(
                [sys.executable, src, d],
                check=True,
                env=env,
                timeout=900,
                capture_output=True,
            )
            ids = np.load(os.path.join(d, "ids.npy"))
            w = np.load(os.path.join(d, "w.npy"))
            return ids, w
    except Exception:
        # numpy fallback replicating jax.lax.top_k tie semantics (lower
        # index wins on equal values).
        s = x.reshape(-1, DIM).astype(np.float32) @ w_router.astype(np.float32).T
        idx = np.argsort(-s, axis=-1, kind="stable")[:, :K]
        c = np.take_along_axis(s, idx, axis=-1)
        e = np.exp(c - c.max(-1, keepdims=True))
        w = e / e.sum(-1, keepdims=True)
        return (
            idx.reshape(B, S, K).astype(np.int32),
            w.reshape(B, S, K).astype(np.float32),
        )


def _q8(v, scale):
    return np.clip(v * scale, -240.0, 240.0).astype(F8)


def _dr_chunks(C):
    """Balanced column chunks of <=512 (fp8 DoubleRow moving free dim is
    2*n <= 1024; PSUM bank holds 512 fp32)."""
    n_ch = max(1, -(-C // 512))
    base, rem = divmod(C, n_ch)
    out, o = [], 0
    for i in range(n_ch):
        n = base + (1 if i < rem else 0)
        out.append((o, o + n))
        o += n
    return out


def _build(C, chunks):
    import concourse.bacc as bacc
    import concourse.tile as tile
    import concourse.mybir as mybir

    f32 = mybir.dt.float32
    f16 = mybir.dt.float16
    f8 = mybir.dt.float8e4
    u8 = mybir.dt.uint8
    SIGMOID = mybir.ActivationFunctionType.Sigmoid
    COPY = mybir.ActivationFunctionType.Copy
    DR = mybir.MatmulPerfMode.DoubleRow

    CP = -(-C // 16) * 16        # fp8 tile stride, 16B-aligned
    drch = _dr_chunks(C)

    # SBUF guard for very skewed routing (C is ~547 for the reference data).
    big = C > 900
    WT_BUFS = 3 if big else 6
    WD_BUFS = 2 if big else 5
    WP_BUFS = 8 if big else 20
    OT_BUFS = 4 if big else 12

    nc = bacc.Bacc("TRN2", target_bir_lowering=False, debug=False)

    # fp8 payloads are staged as uint8 and bitcast on-device (the axon PJRT
    # input path does not handle float8 arrays)
    x8d = nc.dram_tensor("x8", [128, KD, CP], u8, kind="ExternalInput").ap()
    xsd = nc.dram_tensor("xs", [KD, 128, C], f16, kind="ExternalInput").ap()
    gd = nc.dram_tensor("g", [128, C], f16, kind="ExternalInput").ap()
    wud = nc.dram_tensor("wup", [NQ, 128, 2, KP, 2, 128], u8, kind="ExternalInput").ap()
    wdd = nc.dram_tensor("wdn", [KD, 128, NP2, 2, 128], u8, kind="ExternalInput").ap()
    # wproj pre-tiled as 64 pairs of v-tiles: one [128, 2048] DMA = 2 v-tiles
    wpd = nc.dram_tensor("wpj", [NV // 2, 128, 2 * DIM], f16, kind="ExternalInput").ap()
    ld = nc.dram_tensor("L", [VOCAB, C], f16, kind="ExternalOutput").ap()

    with tile.TileContext(nc) as tc:
        with (
            tc.tile_pool(name="persist", bufs=1) as per,
            tc.tile_pool(name="wpool", bufs=1) as wpool,
            tc.tile_pool(name="tpool", bufs=1) as tpool,
            tc.tile_pool(name="pspool", bufs=1, space="PSUM") as ps,
        ):
            # first phase-1 weight tile + X8 go first, split into per-piece
            # DMAs so they spread over many queues (a single large DMA rides
            # one queue and gates the first matmul by ~10us)
            wt0 = wpool.tile([128, 2, KP, 2, 128], u8, name="wt", tag="wt", bufs=WT_BUFS)
            nc.sync.dma_start(wt0[:, 0], wud[0, :, 0])
            nc.scalar.dma_start(wt0[:, 1], wud[0, :, 1])
            X8 = per.tile([128, KD, CP], u8, name="X8")
            nc.sync.dma_start(X8[:, :2], x8d[:, :2])
            nc.scalar.dma_start(X8[:, 2:4], x8d[:, 2:4])
            nc.sync.dma_start(X8[:, 4:6], x8d[:, 4:6])
            nc.scalar.dma_start(X8[:, 6:], x8d[:, 6:])
            X8f = X8[:].bitcast(f8)
            Xs = [per.tile([128, C], f16, name=f"Xs{j}") for j in range(KD)]
            G = per.tile([128, C], f16, name="G")
            A8 = per.tile([128, NQ, CP], f8, name="A8")
            A8f = A8[:]
            Y = [per.tile([128, C], f16, name=f"Y{j}") for j in range(KD)]
            zb = per.tile([128, 1], f32, name="zb")
            nc.gpsimd.memset(zb[:], 0.0)

            def bank(nm):
                return ps.tile([128, 512], f32, name=nm, tag="bank", bufs=8)

            # phase 1: psum = Wup8.T @ X8 per 128-row block (lin & gate),
            # A8 = 2^4 * lin * silu(gate); DoubleRow k=256 per pass, the
            # stationary streams all chunks before switching (hides the
            # 256-col LDWEIGHTS).
            for q in range(NQ):
                if q == 0:
                    wt = wt0
                else:
                    wt = wpool.tile(
                        [128, 2, KP, 2, 128], u8, name="wt", tag="wt", bufs=WT_BUFS
                    )
                    for w in range(2):
                        nc.sync.dma_start(wt[:, w], wud[q, :, w])
                wtf = wt[:].bitcast(f8)
                pls = [bank("pl") for _ in drch]
                pgs = [bank("pg") for _ in drch]
                for t in range(KP):
                    for ci, (c0, c1) in enumerate(drch):
                        n = c1 - c0
                        rhs = X8f[:, 2 * t : 2 * t + 2, c0:c1]
                        nc.tensor.matmul(
                            pls[ci][:, :n], wtf[:, 0, t], rhs,
                            start=(t == 0), stop=(t == KP - 1), perf_mode=DR,
                        )
                        nc.tensor.matmul(
                            pgs[ci][:, :n], wtf[:, 1, t], rhs,
                            start=(t == 0), stop=(t == KP - 1), perf_mode=DR,
                        )
                for ci, (c0, c1) in enumerate(drch):
                    n = c1 - c0
                    st = tpool.tile([128, 512], f32, name="st", tag="st", bufs=3)
                    plc = tpool.tile([128, 512], f32, name="plc", tag="plc", bufs=3)
                    nc.scalar.activation(
                        st[:, :n], pgs[ci][:, :n], SIGMOID, bias=zb[:], scale=SIG_SCALE
                    )
                    nc.scalar.activation(
                        plc[:, :n], pls[ci][:, :n], COPY, bias=0.0, scale=PLC_SCALE
                    )
                    nc.vector.tensor_mul(st[:, :n], st[:, :n], pgs[ci][:, :n])
                    nc.vector.tensor_mul(A8[:, q, c0:c1], plc[:, :n], st[:, :n])

            # phase 2: psum = Wdn8.T @ A8 (11 DoubleRow k-pairs),
            # Y = (psum + 2^10 x) * (2^-10 g)
            for j in range(KD):
                nc.sync.dma_start(Xs[j][:], xsd[j])
            nc.sync.dma_start(G[:], gd)
            for m in range(KD):
                wd = wpool.tile(
                    [128, NP2, 2, 128], u8, name="wd", tag="wd", bufs=WD_BUFS
                )
                half = NP2 // 2
                nc.sync.dma_start(wd[:, :half], wdd[m, :, :half])
                nc.sync.dma_start(wd[:, half:], wdd[m, :, half:])
                wdf = wd[:].bitcast(f8)
                pys = [bank("py") for _ in drch]
                for t in range(NP2):
                    for ci, (c0, c1) in enumerate(drch):
                        n = c1 - c0
                        nc.tensor.matmul(
                            pys[ci][:, :n], wdf[:, t], A8f[:, 2 * t : 2 * t + 2, c0:c1],
                            start=(t == 0), stop=(t == NP2 - 1), perf_mode=DR,
                        )
                for ci, (c0, c1) in enumerate(drch):
                    n = c1 - c0
                    nc.vector.tensor_add(
                        pys[ci][:, :n], pys[ci][:, :n], Xs[m][:, c0:c1]
                    )
                    nc.vector.tensor_mul(Y[m][:, c0:c1], pys[ci][:, :n], G[:, c0:c1])

            # phase 3 (fp16): L = Wproj @ Y, two v-tiles per weight DMA, one
            # out tile + DMA per v-tile (both chunks batched)
            for vp in range(NV // 2):
                wp = wpool.tile([128, 2 * DIM], f16, name="wp", tag="wp", bufs=WP_BUFS)
                nc.sync.dma_start(wp[:, :DIM], wpd[vp, :, :DIM])
                nc.sync.dma_start(wp[:, DIM:], wpd[vp, :, DIM:])
                for h in range(2):
                    v = 2 * vp + h
                    wv = wp[:, h * DIM : (h + 1) * DIM]
                    ot = tpool.tile([128, C], f16, name="ot", tag="ot", bufs=OT_BUFS)
                    for (c0, c1) in chunks:
                        n = c1 - c0
                        pL = bank("pL")
                        for j in range(KD):
                            nc.tensor.matmul(
                                pL[:, :n], wv[:, j * 128 : (j + 1) * 128],
                                Y[j][:, c0:c1],
                                start=(j == 0), stop=(j == KD - 1),
                            )
                        nc.vector.tensor_copy(ot[:, c0:c1], pL[:, :n])
                    eng = nc.scalar if h == 0 else nc.sync
                    eng.dma_start(ld[v * 128 : (v + 1) * 128, :], ot[:])

    nc.compile()
    return nc


def _prep_core_inputs(e, x_flat, w_up, w_down, w_proj, tok, gates, C):
    cnt = len(tok)
    CP = -(-C // 16) * 16

    Xp = np.zeros((C, DIM), np.float32)
    if cnt:
        Xp[:cnt] = x_flat[tok]
    xT = np.ascontiguousarray(Xp.T)                          # [DIM, C]
    # x8: [128p, KD, CP] with [p, j, c] = 2^4 * x[c, j*128+p]
    x8 = np.zeros((128, KD, CP), F8)
    x8[:, :, :C] = _q8(xT.reshape(KD, 128, C).transpose(1, 0, 2), S_X)
    # xs: [KD, 128, C] fp16 = 2^10 * x in feature-major tiles (residual)
    xs = (xT.reshape(KD, 128, C) * P2_SCALE).astype(np.float16)

    g = np.zeros((C,), np.float32)
    if cnt:
        g[:cnt] = gates
    gb = np.ascontiguousarray(
        np.broadcast_to(g / P2_SCALE, (128, C))
    ).astype(np.float16)

    wu = np.asarray(w_up[e], np.float32)
    lin = np.zeros((HID_P, DIM), np.float32)
    lin[:HIDDEN] = wu[:HIDDEN]
    gat = np.zeros((HID_P, DIM), np.float32)
    gat[:HIDDEN] = wu[HIDDEN : 2 * HIDDEN]
    # [q, p, which, t, i, c] = W[q*128 + c, (2t+i)*128 + p]
    def dr_up(m):
        r = m.reshape(NQ, 128, KP, 2, 128)                   # [q, c, t, i, p]
        return r.transpose(0, 4, 2, 3, 1)                    # [q, p, t, i, c]

    wup8 = np.empty((NQ, 128, 2, KP, 2, 128), F8)
    wup8[:, :, 0] = _q8(dr_up(lin), S_WU)
    wup8[:, :, 1] = _q8(dr_up(gat), S_WU)

    wdt = np.zeros((HID_P, DIM), np.float32)
    wdt[:HIDDEN] = np.asarray(w_down[e], np.float32).T       # [k, m_col]
    # [m, p, t, i, c] = wdn[m*128 + c, (2t+i)*128 + p] = wdt[(2t+i)*128+p, ...]
    wdn8 = _q8(
        wdt.reshape(NP2, 2, 128, KD, 128).transpose(3, 2, 0, 1, 4), S_WD
    )
    wdn8 = np.ascontiguousarray(wdn8)

    wpj_t = (
        np.ascontiguousarray(
            np.asarray(w_proj[e], np.float32)
            .reshape(NV, 128, KD, 128)
            .transpose(0, 3, 2, 1)
        )
        .astype(np.float16)
        .reshape(NV // 2, 2, 128, DIM)
        .transpose(0, 2, 1, 3)
        .reshape(NV // 2, 128, 2 * DIM)
    )
    wpj_t = np.ascontiguousarray(wpj_t)

    return {
        "x8": x8.view(np.uint8),
        "xs": np.ascontiguousarray(xs),
        "g": gb,
        "wup": np.ascontiguousarray(wup8).view(np.uint8),
        "wdn": wdn8.view(np.uint8),
        "wpj": wpj_t,
    }


_last_results = None  # for test harness inspection (exec_time_ns etc.)


def kernel(x, w_router, w_up, w_down, w_proj):
    global _last_results
    x = np.asarray(x, np.float32)

    ids, wsm = _route(x, w_router)
    ids_flat = ids.reshape(-1).astype(np.int64)
    w_flat = wsm.reshape(-1).astype(np.float32)
    order = np.argsort(ids_flat, kind="stable")
    counts = np.bincount(ids_flat, minlength=N_EXPERTS)
    offs = np.concatenate([[0], np.cumsum(counts)])

    C = int(counts.max())
    n_ch = max(1, -(-C // 512))
    base, rem = divmod(C, n_ch)
    sizes = [base + (1 if i < rem else 0) for i in range(n_ch)]
    chunks = []
    o = 0
    for s_ in sizes:
        chunks.append((o, o + s_))
        o += s_

    x_flat = x.reshape(B * S, DIM)
    in_maps = []
    tok_lists = []
    for e in range(N_EXPERTS):
        rows = order[offs[e] : offs[e + 1]]
        tok = rows // K
        tok_lists.append(tok)
        in_maps.append(
            _prep_core_inputs(e, x_flat, w_up, w_down, w_proj, tok, w_flat[rows], C)
        )

    nc = _build(C, chunks)

    from concourse.bass_utils import run_bass_kernel_spmd

    trace = bool(int(os.environ.get("MOE_KERNEL_TRACE", "0")))
    kw = {}
    if trace:
        kw["trace"] = True
        kw["trace_cores"] = list(range(N_CORES))
    res = run_bass_kernel_spmd(nc, in_maps, list(range(N_CORES)), **kw)
    _last_results = res

    out_flat = np.zeros((B * S, VOCAB), np.float32)
    for e in range(N_EXPERTS):
        tok = tok_lists[e]
        cnt = len(tok)
        if cnt:
            out_flat[tok] += res.results[e]["L"][:, :cnt].T.astype(np.float32)
    return out_flat.reshape(B, S, VOCAB)


# revision 24
# speedup vs baseline: 1.0700x; 1.0700x over previous
"""MoE head (top-2 routing, swiglu MLP + vocab projection) on 8 Trainium2 cores.

Expert-parallel: one expert per NeuronCore. Routing (tiny: router scores +
top-k + stable dispatch sort) is replicated bitwise on host CPU with jax-cpu
(matching the fp32 reference); each core runs its expert's full MLP + vocab
projection over that expert's tokens.

Device layout: every matmul keeps a weight tile stationary and streams
activations as the moving operand; activations live as [feature, token] so no
on-chip transposes are needed anywhere; the top-2 gate is folded into the
activations before the vocab projection, so the host combine is a pure
scatter-add.

Phases 1 (up-proj + swiglu) and 2 (down-proj + residual) run in fp8 e4m3 with
MatmulPerfMode.DoubleRow (128x256 virtual PE array, ~1.5-1.8x the fp16 matmul
rate); their quantization error lands on y *before* the vocab projection and
is attenuated by it, staying well inside the correctness budget. Phase 3
(vocab projection, 2/3 of the FLOPs) stays fp16: a single-pass fp8 vocab
matmul puts ~2.7e-2 of direct error on the logits (budget 2e-2), and
compensated multi-pass fp8 is no faster than fp16's FWL path.

Scale algebra (all powers of two, exact):
  X8 = 2^4 x, Wup8 = 2^6 wup          -> psum p = 2^10 h
  sigmoid(2^-10 p) = sigmoid(h_gate); plc = 2^-16 p_lin = 2^-6 h_lin
  A8 = plc * (sig * p_gate) = 2^4 a   (|A8| <~ 90 < 240, no fp8 overflow)
  Wdn8 = 2^6 wdn                      -> psum = 2^10 (y - x)
  X16s = 2^10 x (fp16), Gs = 2^-10 g  -> Y16 = (psum + X16s) * Gs = y*g
"""

import os
import sys
import subprocess
import tempfile

import numpy as np
import ml_dtypes

for _p in ("/opt/trn_rl_repo",):
    if os.path.isdir(_p) and _p not in sys.path:
        sys.path.insert(0, _p)

B, S, DIM = 2, 1024, 1024
N_EXPERTS, K = 8, 2
VOCAB = 16384
HIDDEN = DIM * 8 // 3            # 2730
HID_P = 2816                     # HIDDEN padded to 22*128
NQ = HID_P // 128                # 22 row tiles of lin/gate; phase-2 k-tiles
NP2 = NQ // 2                    # 11 phase-2 DoubleRow k-pairs
KD = DIM // 128                  # 8
KP = KD // 2                     # 4 phase-1/3 k-pairs
NV = VOCAB // 128                # 128
N_CORES = 8

F8 = ml_dtypes.float8_e4m3       # TRN FP8_EXP4 (max normal 240)
S_X = 16.0                       # 2^4
S_WU = 64.0                      # 2^6
S_A = 16.0                       # 2^4
S_WD = 64.0                      # 2^6
SIG_SCALE = 1.0 / 1024.0         # 2^-10 = 1/(S_X*S_WU)
PLC_SCALE = 1.0 / 65536.0        # 2^-16 = S_A/(S_X*S_WU)^2
P2_SCALE = 1024.0                # 2^10 = S_A*S_WD

# Routing must make the same discrete top-k choices as the reference, which
# runs under jax on CPU; replicate it in a JAX_PLATFORMS=cpu subprocess
# (this process's jax backend is the axon/trn2 platform).
def _cpu_jax_env():
    env = dict(os.environ)
    env.pop("TRN_TERMINAL_POOL_IPS", None)
    env["JAX_PLATFORMS"] = "cpu"
    try:
        import jax

        sp = os.path.dirname(os.path.dirname(jax.__file__))
        env["PYTHONPATH"] = sp + os.pathsep + env.get("PYTHONPATH", "")
    except Exception:
        pass
    return env


_ROUTE_SRC = r"""
import os, sys
os.environ["JAX_PLATFORMS"] = "cpu"
import numpy as np
d = sys.argv[1]
x = np.load(os.path.join(d, "x.npy"))
wr = np.load(os.path.join(d, "wr.npy"))
import jax, jax.numpy as jnp
scores = jnp.einsum("bsd,nd->bsn", jnp.asarray(x), jnp.asarray(wr))
c, ids = jax.lax.top_k(scores, 2)
w = jax.nn.softmax(c, axis=-1)
np.save(os.path.join(d, "ids.npy"), np.asarray(ids))
np.save(os.path.join(d, "w.npy"), np.asarray(w, dtype=np.float32))
"""


def _route(x, w_router):
    try:
        with tempfile.TemporaryDirectory() as d:
            np.save(os.path.join(d, "x.npy"), np.asarray(x, np.float32))
            np.save(os.path.join(d, "wr.npy"), np.asarray(w_router, np.float32))
            src = os.path.join(d, "route.py")
            with open(src, "w") as f:
                f.write(_ROUTE_SRC)
            env = _cpu_jax_env()
            subprocess.run(
                [sys.executable, src, d],
                check=True,
                env=env,
                timeout=900,
                capture_output=True,
            )
            ids = np.load(os.path.join(d, "ids.npy"))
            w = np.load(os.path.join(d, "w.npy"))
            return ids, w
    except Exception:
        # numpy fallback replicating jax.lax.top_k tie semantics (lower
        # index wins on equal values).
        s = x.reshape(-1, DIM).astype(np.float32) @ w_router.astype(np.float32).T
        idx = np.argsort(-s, axis=-1, kind="stable")[:, :K]
        c = np.take_along_axis(s, idx, axis=-1)
        e = np.exp(c - c.max(-1, keepdims=True))
        w = e / e.sum(-1, keepdims=True)
        return (
            idx.reshape(B, S, K).astype(np.int32),
            w.reshape(B, S, K).astype(np.float32),
        )


def _q8(v, scale):
    return np.clip(v * scale, -240.0, 240.0).astype(F8)


def _dr_chunks(C):
    """Balanced column chunks of <=512 (fp8 DoubleRow moving free dim is
    2*n <= 1024; PSUM bank holds 512 fp32)."""
    n_ch = max(1, -(-C // 512))
    base, rem = divmod(C, n_ch)
    out, o = [], 0
    for i in range(n_ch):
        n = base + (1 if i < rem else 0)
        out.append((o, o + n))
        o += n
    return out


def _build(C, chunks):
    import concourse.bacc as bacc
    import concourse.tile as tile
    import concourse.mybir as mybir

    f32 = mybir.dt.float32
    f16 = mybir.dt.float16
    f8 = mybir.dt.float8e4
    u8 = mybir.dt.uint8
    SIGMOID = mybir.ActivationFunctionType.Sigmoid
    COPY = mybir.ActivationFunctionType.Copy
    DR = mybir.MatmulPerfMode.DoubleRow

    CP = -(-C // 16) * 16        # fp8 tile stride, 16B-aligned
    drch = _dr_chunks(C)

    # SBUF guard for very skewed routing (C is ~547 for the reference data).
    big = C > 900
    WT_BUFS = 3 if big else 6
    WD_BUFS = 2 if big else 5
    WP_BUFS = 8 if big else 16
    OT_BUFS = 4 if big else 10

    nc = bacc.Bacc("TRN2", target_bir_lowering=False, debug=False)

    # fp8 payloads are staged as uint8 and bitcast on-device (the axon PJRT
    # input path does not handle float8 arrays)
    x8d = nc.dram_tensor("x8", [128, KD, CP], u8, kind="ExternalInput").ap()
    xsd = nc.dram_tensor("xs", [KD, 128, C], f16, kind="ExternalInput").ap()
    gd = nc.dram_tensor("g", [128, C], f16, kind="ExternalInput").ap()
    wud = nc.dram_tensor("wup", [NQ, 128, 2, KP, 2, 128], u8, kind="ExternalInput").ap()
    wdd = nc.dram_tensor("wdn", [KD, 128, NP2, 2, 128], u8, kind="ExternalInput").ap()
    # wproj pre-tiled as 64 pairs of v-tiles: one [128, 2048] DMA = 2 v-tiles
    wpd = nc.dram_tensor("wpj", [NV // 2, 128, 2 * DIM], f16, kind="ExternalInput").ap()
    ld = nc.dram_tensor("L", [VOCAB, C], f16, kind="ExternalOutput").ap()

    with tile.TileContext(nc) as tc:
        with (
            tc.tile_pool(name="persist", bufs=1) as per,
            tc.tile_pool(name="wpool", bufs=1) as wpool,
            tc.tile_pool(name="tpool", bufs=1) as tpool,
            tc.tile_pool(name="pspool", bufs=1, space="PSUM") as ps,
        ):
            # first phase-1 weight tile + X8 go first, split into per-piece
            # DMAs so they spread over many queues (a single large DMA rides
            # one queue and gates the first matmul by ~10us)
            wt0 = wpool.tile([128, 2, KP, 2, 128], u8, name="wt", tag="wt", bufs=WT_BUFS)
            nc.sync.dma_start(wt0[:, 0], wud[0, :, 0])
            nc.scalar.dma_start(wt0[:, 1], wud[0, :, 1])
            X8 = per.tile([128, KD, CP], u8, name="X8")
            nc.sync.dma_start(X8[:, : KD // 2], x8d[:, : KD // 2])
            nc.scalar.dma_start(X8[:, KD // 2 :], x8d[:, KD // 2 :])
            X8f = X8[:].bitcast(f8)
            Xs = [per.tile([128, C], f16, name=f"Xs{j}") for j in range(KD)]
            G = per.tile([128, C], f16, name="G")
            A8 = per.tile([128, NQ, CP], f8, name="A8")
            A8f = A8[:]
            Y = [per.tile([128, C], f16, name=f"Y{j}") for j in range(KD)]
            zb = per.tile([128, 1], f32, name="zb")
            nc.gpsimd.memset(zb[:], 0.0)

            def bank(nm):
                return ps.tile([128, 512], f32, name=nm, tag="bank", bufs=8)

            # phase 1: psum = Wup8.T @ X8 per 128-row block (lin & gate),
            # A8 = 2^4 * lin * silu(gate); DoubleRow k=256 per pass, the
            # stationary streams all chunks before switching (hides the
            # 256-col LDWEIGHTS).
            for q in range(NQ):
                if q == 0:
                    wt = wt0
                else:
                    wt = wpool.tile(
                        [128, 2, KP, 2, 128], u8, name="wt", tag="wt", bufs=WT_BUFS
                    )
                    for w in range(2):
                        nc.sync.dma_start(wt[:, w], wud[q, :, w])
                wtf = wt[:].bitcast(f8)
                pls = [bank("pl") for _ in drch]
                pgs = [bank("pg") for _ in drch]
                for t in range(KP):
                    for ci, (c0, c1) in enumerate(drch):
                        n = c1 - c0
                        rhs = X8f[:, 2 * t : 2 * t + 2, c0:c1]
                        nc.tensor.matmul(
                            pls[ci][:, :n], wtf[:, 0, t], rhs,
                            start=(t == 0), stop=(t == KP - 1), perf_mode=DR,
                        )
                        nc.tensor.matmul(
                            pgs[ci][:, :n], wtf[:, 1, t], rhs,
                            start=(t == 0), stop=(t == KP - 1), perf_mode=DR,
                        )
                for ci, (c0, c1) in enumerate(drch):
                    n = c1 - c0
                    st = tpool.tile([128, 512], f32, name="st", tag="st", bufs=3)
                    plc = tpool.tile([128, 512], f32, name="plc", tag="plc", bufs=3)
                    nc.scalar.activation(
                        st[:, :n], pgs[ci][:, :n], SIGMOID, bias=zb[:], scale=SIG_SCALE
                    )
                    nc.scalar.activation(
                        plc[:, :n], pls[ci][:, :n], COPY, bias=0.0, scale=PLC_SCALE
                    )
                    nc.vector.tensor_mul(st[:, :n], st[:, :n], pgs[ci][:, :n])
                    nc.vector.tensor_mul(A8[:, q, c0:c1], plc[:, :n], st[:, :n])

            # phase 2: psum = Wdn8.T @ A8 (11 DoubleRow k-pairs),
            # Y = (psum + 2^10 x) * (2^-10 g)
            for j in range(KD):
                nc.sync.dma_start(Xs[j][:], xsd[j])
            nc.sync.dma_start(G[:], gd)
            for m in range(KD):
                wd = wpool.tile(
                    [128, NP2, 2, 128], u8, name="wd", tag="wd", bufs=WD_BUFS
                )
                half = NP2 // 2
                nc.sync.dma_start(wd[:, :half], wdd[m, :, :half])
                nc.sync.dma_start(wd[:, half:], wdd[m, :, half:])
                wdf = wd[:].bitcast(f8)
                pys = [bank("py") for _ in drch]
                for t in range(NP2):
                    for ci, (c0, c1) in enumerate(drch):
                        n = c1 - c0
                        nc.tensor.matmul(
                            pys[ci][:, :n], wdf[:, t], A8f[:, 2 * t : 2 * t + 2, c0:c1],
                            start=(t == 0), stop=(t == NP2 - 1), perf_mode=DR,
                        )
                for ci, (c0, c1) in enumerate(drch):
                    n = c1 - c0
                    nc.vector.tensor_add(
                        pys[ci][:, :n], pys[ci][:, :n], Xs[m][:, c0:c1]
                    )
                    nc.vector.tensor_mul(Y[m][:, c0:c1], pys[ci][:, :n], G[:, c0:c1])

            # phase 3 (fp16): L = Wproj @ Y, two v-tiles per weight DMA, one
            # out tile + DMA per v-tile (both chunks batched)
            for vp in range(NV // 2):
                wp = wpool.tile([128, 2 * DIM], f16, name="wp", tag="wp", bufs=WP_BUFS)
                nc.sync.dma_start(wp[:, :DIM], wpd[vp, :, :DIM])
                nc.sync.dma_start(wp[:, DIM:], wpd[vp, :, DIM:])
                for h in range(2):
                    v = 2 * vp + h
                    wv = wp[:, h * DIM : (h + 1) * DIM]
                    ot = tpool.tile([128, C], f16, name="ot", tag="ot", bufs=OT_BUFS)
                    for (c0, c1) in chunks:
                        n = c1 - c0
                        pL = bank("pL")
                        for j in range(KD):
                            nc.tensor.matmul(
                                pL[:, :n], wv[:, j * 128 : (j + 1) * 128],
                                Y[j][:, c0:c1],
                                start=(j == 0), stop=(j == KD - 1),
                            )
                        nc.vector.tensor_copy(ot[:, c0:c1], pL[:, :n])
                    eng = nc.scalar if h == 0 else nc.sync
                    eng.dma_start(ld[v * 128 : (v + 1) * 128, :], ot[:])

    nc.compile()
    return nc


def _prep_core_inputs(e, x_flat, w_up, w_down, w_proj, tok, gates, C):
    cnt = len(tok)
    CP = -(-C // 16) * 16

    Xp = np.zeros((C, DIM), np.float32)
    if cnt:
        Xp[:cnt] = x_flat[tok]
    xT = np.ascontiguousarray(Xp.T)                          # [DIM, C]
    # x8: [128p, KD, CP] with [p, j, c] = 2^4 * x[c, j*128+p]
    x8 = np.zeros((128, KD, CP), F8)
    x8[:, :, :C] = _q8(xT.reshape(KD, 128, C).transpose(1, 0, 2), S_X)
    # xs: [KD, 128, C] fp16 = 2^10 * x in feature-major tiles (residual)
    xs = (xT.reshape(KD, 128, C) * P2_SCALE).astype(np.float16)

    g = np.zeros((C,), np.float32)
    if cnt:
        g[:cnt] = gates
    gb = np.ascontiguousarray(
        np.broadcast_to(g / P2_SCALE, (128, C))
    ).astype(np.float16)

    wu = np.asarray(w_up[e], np.float32)
    lin = np.zeros((HID_P, DIM), np.float32)
    lin[:HIDDEN] = wu[:HIDDEN]
    gat = np.zeros((HID_P, DIM), np.float32)
    gat[:HIDDEN] = wu[HIDDEN : 2 * HIDDEN]
    # [q, p, which, t, i, c] = W[q*128 + c, (2t+i)*128 + p]
    def dr_up(m):
        r = m.reshape(NQ, 128, KP, 2, 128)                   # [q, c, t, i, p]
        return r.transpose(0, 4, 2, 3, 1)                    # [q, p, t, i, c]

    wup8 = np.empty((NQ, 128, 2, KP, 2, 128), F8)
    wup8[:, :, 0] = _q8(dr_up(lin), S_WU)
    wup8[:, :, 1] = _q8(dr_up(gat), S_WU)

    wdt = np.zeros((HID_P, DIM), np.float32)
    wdt[:HIDDEN] = np.asarray(w_down[e], np.float32).T       # [k, m_col]
    # [m, p, t, i, c] = wdn[m*128 + c, (2t+i)*128 + p] = wdt[(2t+i)*128+p, ...]
    wdn8 = _q8(
        wdt.reshape(NP2, 2, 128, KD, 128).transpose(3, 2, 0, 1, 4), S_WD
    )
    wdn8 = np.ascontiguousarray(wdn8)

    wpj_t = (
        np.ascontiguousarray(
            np.asarray(w_proj[e], np.float32)
            .reshape(NV, 128, KD, 128)
            .transpose(0, 3, 2, 1)
        )
        .astype(np.float16)
        .reshape(NV // 2, 2, 128, DIM)
        .transpose(0, 2, 1, 3)
        .reshape(NV // 2, 128, 2 * DIM)
    )
    wpj_t = np.ascontiguousarray(wpj_t)

    return {
        "x8": x8.view(np.uint8),
        "xs": np.ascontiguousarray(xs),
        "g": gb,
        "wup": np.ascontiguousarray(wup8).view(np.uint8),
        "wdn": wdn8.view(np.uint8),
        "wpj": wpj_t,
    }


_last_results = None  # for test harness inspection (exec_time_ns etc.)


def kernel(x, w_router, w_up, w_down, w_proj):
    global _last_results
    x = np.asarray(x, np.float32)

    ids, wsm = _route(x, w_router)
    ids_flat = ids.reshape(-1).astype(np.int64)
    w_flat = wsm.reshape(-1).astype(np.float32)
    order = np.argsort(ids_flat, kind="stable")
    counts = np.bincount(ids_flat, minlength=N_EXPERTS)
    offs = np.concatenate([[0], np.cumsum(counts)])

    C = int(counts.max())
    n_ch = max(1, -(-C // 512))
    base, rem = divmod(C, n_ch)
    sizes = [base + (1 if i < rem else 0) for i in range(n_ch)]
    chunks = []
    o = 0
    for s_ in sizes:
        chunks.append((o, o + s_))
        o += s_

    x_flat = x.reshape(B * S, DIM)
    in_maps = []
    tok_lists = []
    for e in range(N_EXPERTS):
        rows = order[offs[e] : offs[e + 1]]
        tok = rows // K
        tok_lists.append(tok)
        in_maps.append(
            _prep_core_inputs(e, x_flat, w_up, w_down, w_proj, tok, w_flat[rows], C)
        )

    nc = _build(C, chunks)

    from concourse.bass_utils import run_bass_kernel_spmd

    trace = bool(int(os.environ.get("MOE_KERNEL_TRACE", "0")))
    kw = {}
    if trace:
        kw["trace"] = True
        kw["trace_cores"] = list(range(N_CORES))
    res = run_bass_kernel_spmd(nc, in_maps, list(range(N_CORES)), **kw)
    _last_results = res

    out_flat = np.zeros((B * S, VOCAB), np.float32)
    for e in range(N_EXPERTS):
        tok = tok_lists[e]
        cnt = len(tok)
        if cnt:
            out_flat[tok] += res.results[e]["L"][:, :cnt].T.astype(np.float32)
    return out_flat.reshape(B, S, VOCAB)
